# revision 79
# baseline (speedup 1.0000x reference)
"""NT-Xent loss kernel for 8 Trainium2 NeuronCores.

Math (matches the reference):
  Z = concat(z_i, z_j).reshape(8192, 128); r = row-l2-normalize(Z)
  sim = r @ r.T                                  (8192 x 8192)
  row i: S_i   = sum_j exp(2*sim[i, j])          (full row, incl. self)
         d_i   = exp(2*sim[i, i])                (self term)
         p_i   = exp(2*sim[i, pair(i)]),  pair(i) = (i + 4096) % 8192
  loss_i = log(S_i - d_i + p_i) - log(p_i)
  loss   = mean_i(loss_i)

Sharding: rows are split across 8 cores (1024 rows each). Every core gets
the full Z, but ROTATED so its own rows come first; this makes the
self-diagonal land at local columns [0, 1024) and the pair diagonal at
[4096, 5120) on every core, so one SPMD program works for all cores.
Each core emits its 1024 per-row losses; the host sums them (the scalar
all-reduce) and divides by 2N.

Host-side staging: z is supplied pre-rotated AND pre-tiled as
z_sh[p, t*128 + d] = z_rot[t*128 + p, d] so every DMA is fully
contiguous per partition.

Per-core pipeline:
  1. DMA z (fp32) in 8 sub-chunks of 1024 rows.
  2. Row norms on DVE only: square, reduce, rsqrt via Quake seed + 2
     fused Newton steps (no ACT Sqrt -> no activation-table thrash).
  3. Scale rows to unit norm, cast bf16, one batched DMA-xbar transpose
     per sub-chunk into RT[d, 8192].
  4. Main loop: per (2048-col chunk x 128-row block): 4 bf16 matmuls
     (512 cols each) into PSUM, one ACT Exp (scale=2) with fused row-sum
     (accum_out); diag-carrying chunks write exp to SBUF scratch so the
     PSUM slot frees immediately and the diag extraction (multiply by
     identity + reduce on DVE) runs off the critical path.
     Four late non-diag chunks are offloaded from the saturated ACT
     engine to the (by then idle) DVE via a one-pass Schraudolph
     exp2-in-int16 (bf16 bit pattern) + fast 2-byte row-sum; the ~3%
     element error is mean-centered and contributes < 2e-4 to the loss.
  5. Epilogue: S - d + p, Ln, subtract, DMA out [128, 8] losses.
"""

import sys

import numpy as np

sys.path.insert(0, "/opt/trn_rl_repo")

from contextlib import ExitStack  # noqa: E402

import concourse.bass as bass  # noqa: E402
import concourse.tile as tile  # noqa: E402
from concourse import bacc, mybir  # noqa: E402
from concourse.bass_utils import run_bass_kernel_spmd  # noqa: E402

try:
    import ml_dtypes  # noqa: E402

    BF16_NP = ml_dtypes.bfloat16
except ImportError:  # pragma: no cover
    BF16_NP = np.float32

P = 128
N_CORES = 8
NROWS = 8192  # 2N
D = 128
ROWS_PER_CORE = NROWS // N_CORES  # 1024
RB = ROWS_PER_CORE // P  # 8 row blocks per core
G = 4  # column chunk groups (main loop)
CH = NROWS // G  # 2048 columns per chunk
SC = 8  # normalization sub-chunks
SCR = NROWS // SC  # 1024 rows per sub-chunk
TPS = SCR // P  # 8 row-tiles per sub-chunk
MM_N = 512  # matmul moving free dim (one PSUM bank)

# Schraudolph exp2-in-bf16-bits for the DVE-offloaded chunks:
# int16 = sim*A + B; the bits, read as bf16, give exp(2*sim) with ~3%
# element error, mean-centered (B tuned on the real input distribution;
# validated |rel err| of the final loss < 1.3e-4).
A_SCH = 2.0 * 128.0 * 1.4426950408889634  # 2*log2(e)*2^7
B_SCH = 16250.0
# (g, rb) chunks whose exp+row-sum runs on DVE instead of ACT.
# Measured: the main loop is PE-bound (~60us of matmul in a ~65us
# window), so mid-loop offloads don't shorten it. But the TAIL is an ACT
# serial drain (the last fills finish ~2 chunks ahead of ACT): offloading
# the second-to-last chunk lets ACT and DVE drain in parallel.
# Measured: every exp-offload variant was neutral-to-worse, both when the
# loop was PE-bound AND after bf16 inputs made it ACT-bound ({(1,5),(3,5)}
# drew 98.1us vs 96.0/96.2 without — the DVE's 1x PSUM read holds the ring
# slot longer than ACT would, stalling the fills).
V_CHUNKS = frozenset()

F32 = mybir.dt.float32
BF16 = mybir.dt.bfloat16
I16 = mybir.dt.int16
U32 = mybir.dt.uint32
AF = mybir.ActivationFunctionType
OP = mybir.AluOpType
AX = mybir.AxisListType

_CACHE = {}


def _broadcast_last(ap: bass.AP, n: int) -> bass.AP:
    """Append a stride-0 dim of size n to an AP (free-axis broadcast)."""
    return bass.AP(tensor=ap.tensor, offset=ap.offset, ap=[*ap.ap, [0, n]])


def _build_nc():
    nc = bacc.Bacc(
        "TRN2", target_bir_lowering=False, debug=False, num_devices=N_CORES
    )
    # z ships as bf16: halves the input DMA time and makes the square
    # STTs 2-byte eligible; the bf16 quantization of z adds ~1e-5 relative
    # error to the loss (validated; gate is 2e-2).
    z = nc.dram_tensor("z", [P, NROWS], BF16, kind="ExternalInput").ap()
    ident = nc.dram_tensor("ident", [P, P], F32, kind="ExternalInput").ap()
    out = nc.dram_tensor("loss8", [P, RB], F32, kind="ExternalOutput").ap()

    with tile.TileContext(nc) as tc, ExitStack() as ctx:
        zpool = ctx.enter_context(tc.tile_pool(name="zpool", bufs=SC))
        sqpool = ctx.enter_context(tc.tile_pool(name="sqpool", bufs=3))
        znpool = ctx.enter_context(tc.tile_pool(name="znpool", bufs=4))
        small = ctx.enter_context(tc.tile_pool(name="small", bufs=4))
        i16pool = ctx.enter_context(tc.tile_pool(name="i16pool", bufs=2))
        vdpool = ctx.enter_context(tc.tile_pool(name="vdpool", bufs=2))
        singles = ctx.enter_context(tc.tile_pool(name="singles", bufs=1))
        psum = ctx.enter_context(tc.tile_pool(name="psum", bufs=2, space="PSUM"))

        # Persistent transposed normalized representation: RT[d, n]
        rt = singles.tile([P, NROWS], BF16)

        Ssum = singles.tile([P, RB * G], F32)  # per (row, chunk) partial sums
        d8 = singles.tile([P, RB], F32)  # exp(2*self)
        p8 = singles.tile([P, RB], F32)  # exp(2*pair)

        # ---- normalization: 8 pipelined sub-chunks of 1024 rows ----
        # All loads are emitted first so no queue-blocking wait (e.g. an
        # xbar transpose waiting on zn) can delay a later load's dispatch.
        zts = []
        for c in range(SC):
            zt = zpool.tile([P, TPS, D], BF16)
            if c == 0:
                # split the first (critical-path) load into two parallel DMAs
                half = SCR // 2
                # dispatch the critical first load from the Scalar hwdge
                # queue, which finishes engine startup before Sync does
                nc.scalar.dma_start(out=zt[:, : TPS // 2, :], in_=z[:, 0:half])
                nc.scalar.dma_start(out=zt[:, TPS // 2 :, :], in_=z[:, half:SCR])
            else:
                nc.sync.dma_start(out=zt[:], in_=z[:, c * SCR : (c + 1) * SCR])
            zts.append(zt)
        sb_ident = singles.tile([P, P], F32)
        nc.sync.dma_start(out=sb_ident[:], in_=ident)

        for c in range(SC):
            zt = zts[c]
            # fused square + row-sum: per tile one scalar_tensor_tensor with
            # accum_out (out = z*z is scratch, accum = sum over free axis).
            # (A fused 2-long-op variant -- one TT square + one reduce --
            # measured 2.7us SLOWER overall, and splitting sub-chunk 0 into
            # two 512-row units through the transpose measured 15us slower:
            # the fine-grained-but-uniform STT structure is what the
            # scheduler handles best.)
            sq = sqpool.tile([P, TPS, D], BF16)
            ss = small.tile([P, TPS], F32)
            zn = znpool.tile([P, TPS, D], BF16)
            for t in range(TPS):
                nc.vector.scalar_tensor_tensor(
                    out=sq[:, t, :],
                    in0=zt[:, t, :],
                    scalar=1.0,
                    in1=zt[:, t, :],
                    op0=OP.mult,
                    op1=OP.mult,
                    accum_out=ss[:, t : t + 1],
                )

            # u = 1/sqrt(ss).
            # NOTE: an ACT-Sqrt + DVE reciprocal_approx_fast variant for
            # c < 2 (shorter critical chain) drew {98.7, 101.0, 116.2}us
            # across three compiles -- the Tile scheduler is nondeterministic
            # between compiles and that variant is schedule-fragile. The
            # all-Quake form below drew a tight {100.5..100.8}us over four
            # compiles; we ship the low-variance config.
            u = small.tile([P, TPS], F32)
            if c < 2:
                # Sub-chunks 0-1 gate the first matmuls (the whole g=0
                # column range), so their chain must be SHORT: ACT Sqrt
                # (engine idle in the prologue; its in-order queue puts all
                # Sqrts before the first Exp, so at most one extra table
                # load, paid while ACT is idle) + single-op DVE fast
                # reciprocal (~51 ULP) = 2 dependent hops instead of the
                # 8-hop Quake chain whose ~0.5-0.9us/hop semaphore latency
                # delayed the first matmul to ~25us.
                nsq = small.tile([P, TPS], F32)
                nc.scalar.activation(out=nsq[:], in_=ss[:], func=AF.Sqrt)
                nc.vector.reciprocal_approx_fast(out=u[:], in_=nsq[:])
            else:
                # Quake seed + 2 fused Newton iterations on DVE. The scalar
                # ALU promotes to f32, so build the seed as
                # (0xBE6EB3BE - bits) via float mult/add, then integer >>1.
                tmp = small.tile([P, TPS], F32)
                nc.vector.tensor_scalar(
                    tmp[:].bitcast(U32),
                    ss[:].bitcast(U32),
                    -1.0,
                    float(0xBE6EB3BE),
                    OP.mult,
                    OP.add,
                )
                nc.vector.tensor_scalar(
                    u[:].bitcast(U32),
                    tmp[:].bitcast(U32),
                    1,
                    None,
                    OP.logical_shift_right,
                )
                for _ in range(2):
                    # t = (y*y * -0.5) * ss ; y = (t + 1.5) * y
                    nc.vector.tensor_mul(tmp[:], u[:], u[:])
                    nc.vector.scalar_tensor_tensor(
                        out=tmp[:], in0=tmp[:], scalar=-0.5, in1=ss[:],
                        op0=OP.mult, op1=OP.mult,
                    )
                    nc.vector.scalar_tensor_tensor(
                        out=u[:], in0=tmp[:], scalar=1.5, in1=u[:],
                        op0=OP.add, op1=OP.mult,
                    )

            nc.vector.tensor_mul(zn[:], zt[:], _broadcast_last(u[:], D))
            # batched xbar transpose for the whole 1024-col sub-chunk:
            # out[a, b, c] = in[c, b*128 + a]  ->  rt[d, t*128+p] = zn[p, t, d]
            nc.sync.dma_start(
                out=rt[:, c * SCR : (c + 1) * SCR].rearrange(
                    "d (t p) -> d t p", p=P
                ),
                in_=zn[:],
                transpose=True,
            )

        # ---- PE warm-up ----
        # The PE sits idle for the whole ~24us normalization prologue, and
        # the power governor then takes ~30us of wall time to ramp the
        # array from ~634ns to its sustained ~376ns per 512-col matmul
        # (~8us lost across the first ~60 real matmuls). Feed it dummy
        # fp32 ident x ident matmuls (ready as soon as the ident DMA lands
        # at ~8.5us) so the real stream starts warm. They finish before
        # rt sub-chunk 0 is transposed, so they delay nothing.
        for _ in range(28):
            pw = psum.tile([P, CH], F32, name="ps")
            nc.tensor.matmul(
                pw[:, 0:P], sb_ident[:], sb_ident[:], start=True, stop=True
            )

        # ---- main loop: sim chunk -> exp -> row-sum ----
        expool = ctx.enter_context(tc.tile_pool(name="expool", bufs=6))
        for g in range(G):
            for rb in range(RB):
                ps = psum.tile([P, CH], F32)
                for s in range(CH // MM_N):
                    nc.tensor.matmul(
                        ps[:, s * MM_N : (s + 1) * MM_N],
                        rt[:, rb * P : (rb + 1) * P],
                        rt[:, g * CH + s * MM_N : g * CH + (s + 1) * MM_N],
                        start=True,
                        stop=True,
                    )
                if g in (0, 2):
                    # chunks carrying the self/pair diagonal: exp to SBUF
                    # scratch (PSUM frees immediately; extraction decouples).
                    # NO accum_out: the kernel is ACT-bound, and each
                    # accum_out costs a flat ~300ns ACTIVATION_READ_
                    # ACCUMULATOR on the critical ACT stream. Since the exp
                    # already lands in SBUF here, the row-sum runs on the
                    # mid-loop-idle DVE instead (all-SBUF fast path).
                    ex = expool.tile([P, CH], F32)
                    nc.scalar.activation(
                        out=ex[:], in_=ps[:], func=AF.Exp, scale=2.0
                    )
                    # diag extract off the critical path: multiply the
                    # 128x128 diag block by identity, reduce along free
                    dst = (d8 if g == 0 else p8)[:, rb : rb + 1]
                    dummy = small.tile([P, P], F32)
                    nc.vector.tensor_mul(
                        dummy[:], ex[:, rb * P : rb * P + P], sb_ident[:]
                    )
                    nc.vector.tensor_reduce(dst, dummy[:], axis=AX.X, op=OP.add)
                    # DVE row-sum of the exp'd chunk ((x*0)+x with accum)
                    exs = vdpool.tile([P, CH], BF16)
                    nc.vector.scalar_tensor_tensor(
                        out=exs[:], in0=ex[:], scalar=0.0, in1=ex[:],
                        op0=OP.mult, op1=OP.add,
                        accum_out=Ssum[:, rb * G + g : rb * G + g + 1],
                    )
                elif (g, rb) in V_CHUNKS:
                    # DVE-offloaded exp: one tensor_scalar converts the
                    # whole chunk to bf16-bit exp2 int16s (frees the PSUM
                    # slot); the 2-byte row-sum runs after the main loop,
                    # hidden under the remaining ACT chunks.
                    it16 = i16pool.tile([P, CH], I16)
                    nc.vector.tensor_scalar(
                        it16[:], ps[:], A_SCH, B_SCH, OP.mult, OP.add
                    )
                    # row-sum via STT-with-accum ((x*0)+x): 2-byte fast
                    # path (TensorReduce always runs 1x). Inline: it reads
                    # SBUF, so the PSUM slot is already free, and the DVE
                    # has slack here.
                    vd = vdpool.tile([P, CH], BF16)
                    nc.vector.scalar_tensor_tensor(
                        out=vd[:], in0=it16[:].bitcast(BF16), scalar=0.0,
                        in1=it16[:].bitcast(BF16), op0=OP.mult, op1=OP.add,
                        accum_out=Ssum[:, rb * G + g : rb * G + g + 1],
                    )
                else:
                    nc.scalar.activation(
                        out=ps[:],
                        in_=ps[:],
                        func=AF.Exp,
                        scale=2.0,
                        accum_out=Ssum[:, rb * G + g : rb * G + g + 1],
                    )

        # ---- epilogue ----
        # (A split two-column-group variant with a fused ln(S'/p) -- meant
        # to overlap rb0-5's reduction chain with the ACT drain -- drew
        # {98.5, 99.7}us vs this form's {97.8, 97.8}us. Reverted.)
        S8 = singles.tile([P, RB], F32)
        nc.vector.tensor_reduce(
            S8[:], Ssum[:].rearrange("p (r g) -> p r g", g=G), axis=AX.X, op=OP.add
        )
        # S8 <- S8 - d8 + p8
        nc.vector.scalar_tensor_tensor(
            out=S8[:], in0=d8[:], scalar=-1.0, in1=S8[:], op0=OP.mult, op1=OP.add
        )
        nc.vector.tensor_add(S8[:], S8[:], p8[:])
        lse = singles.tile([P, RB], F32)
        nc.scalar.activation(out=lse[:], in_=S8[:], func=AF.Ln)
        p2 = singles.tile([P, RB], F32)
        nc.scalar.activation(out=p2[:], in_=p8[:], func=AF.Ln)
        loss8 = singles.tile([P, RB], F32)
        nc.vector.scalar_tensor_tensor(
            out=loss8[:], in0=p2[:], scalar=-1.0, in1=lse[:], op0=OP.mult, op1=OP.add
        )
        nc.sync.dma_start(out=out, in_=loss8[:])

    nc.compile()
    return nc


def get_nc():
    if "nc" not in _CACHE:
        _CACHE["nc"] = _build_nc()
    return _CACHE["nc"]


def make_in_maps(z_i: np.ndarray, z_j: np.ndarray):
    Z = np.concatenate(
        [
            np.asarray(z_i, np.float32).reshape(NROWS // 2, D),
            np.asarray(z_j, np.float32).reshape(NROWS // 2, D),
        ],
        axis=0,
    )
    ident = np.eye(P, dtype=np.float32)
    in_maps = []
    for k in range(N_CORES):
        zk = np.roll(Z, -k * ROWS_PER_CORE, axis=0)
        # z_sh[p, t*128+d] = zk[t*128+p, d]: contiguous per-partition DMA
        zsh = np.ascontiguousarray(
            zk.reshape(NROWS // P, P, D).transpose(1, 0, 2)
        ).reshape(P, NROWS).astype(BF16_NP)
        in_maps.append({"z": zsh, "ident": ident})
    return in_maps


def run_full(z_i: np.ndarray, z_j: np.ndarray, trace: bool = False):
    nc = get_nc()
    in_maps = make_in_maps(z_i, z_j)
    res = run_bass_kernel_spmd(nc, in_maps, list(range(N_CORES)), trace=trace)
    total = 0.0
    for k in range(N_CORES):
        total += float(np.asarray(res.results[k]["loss8"], np.float64).sum())
    loss = np.float32(total / NROWS)
    return loss, res


def kernel(z_i: np.ndarray, z_j: np.ndarray) -> np.ndarray:
    loss, _ = run_full(z_i, z_j, trace=False)
    return np.asarray(loss, dtype=np.float32)


# revision 80
# speedup vs baseline: 1.0469x; 1.0469x over previous
"""NT-Xent loss kernel for 8 Trainium2 NeuronCores.

Math (matches the reference):
  Z = concat(z_i, z_j).reshape(8192, 128); r = row-l2-normalize(Z)
  sim = r @ r.T                                  (8192 x 8192)
  row i: S_i   = sum_j exp(2*sim[i, j])          (full row, incl. self)
         d_i   = exp(2*sim[i, i])                (self term)
         p_i   = exp(2*sim[i, pair(i)]),  pair(i) = (i + 4096) % 8192
  loss_i = log(S_i - d_i + p_i) - log(p_i)
  loss   = mean_i(loss_i)

Sharding: rows are split across 8 cores (1024 rows each). Every core gets
the full Z, but ROTATED so its own rows come first; this makes the
self-diagonal land at local columns [0, 1024) and the pair diagonal at
[4096, 5120) on every core, so one SPMD program works for all cores.
Each core emits its 1024 per-row losses; the host sums them (the scalar
all-reduce) and divides by 2N.

Host-side staging: z is supplied pre-rotated AND pre-tiled as
z_sh[p, t*128 + d] = z_rot[t*128 + p, d] so every DMA is fully
contiguous per partition.

Per-core pipeline:
  1. DMA z (fp32) in 8 sub-chunks of 1024 rows.
  2. Row norms on DVE only: square, reduce, rsqrt via Quake seed + 2
     fused Newton steps (no ACT Sqrt -> no activation-table thrash).
  3. Scale rows to unit norm, cast bf16, one batched DMA-xbar transpose
     per sub-chunk into RT[d, 8192].
  4. Main loop: per (2048-col chunk x 128-row block): 4 bf16 matmuls
     (512 cols each) into PSUM, one ACT Exp (scale=2) with fused row-sum
     (accum_out); diag-carrying chunks write exp to SBUF scratch so the
     PSUM slot frees immediately and the diag extraction (multiply by
     identity + reduce on DVE) runs off the critical path.
     Four late non-diag chunks are offloaded from the saturated ACT
     engine to the (by then idle) DVE via a one-pass Schraudolph
     exp2-in-int16 (bf16 bit pattern) + fast 2-byte row-sum; the ~3%
     element error is mean-centered and contributes < 2e-4 to the loss.
  5. Epilogue: S - d + p, Ln, subtract, DMA out [128, 8] losses.
"""

import sys

import numpy as np

sys.path.insert(0, "/opt/trn_rl_repo")

from contextlib import ExitStack  # noqa: E402

import concourse.bass as bass  # noqa: E402
import concourse.tile as tile  # noqa: E402
from concourse import bacc, mybir  # noqa: E402
from concourse.bass_utils import run_bass_kernel_spmd  # noqa: E402

try:
    import ml_dtypes  # noqa: E402

    BF16_NP = ml_dtypes.bfloat16
except ImportError:  # pragma: no cover
    BF16_NP = np.float32

P = 128
N_CORES = 8
NROWS = 8192  # 2N
D = 128
ROWS_PER_CORE = NROWS // N_CORES  # 1024
RB = ROWS_PER_CORE // P  # 8 row blocks per core
G = 4  # column chunk groups (main loop)
CH = NROWS // G  # 2048 columns per chunk
SC = 8  # normalization sub-chunks
SCR = NROWS // SC  # 1024 rows per sub-chunk
TPS = SCR // P  # 8 row-tiles per sub-chunk
MM_N = 512  # matmul moving free dim (one PSUM bank)

# Schraudolph exp2-in-bf16-bits for the DVE-offloaded chunks:
# int16 = sim*A + B; the bits, read as bf16, give exp(2*sim) with ~3%
# element error, mean-centered (B tuned on the real input distribution;
# validated |rel err| of the final loss < 1.3e-4).
A_SCH = 2.0 * 128.0 * 1.4426950408889634  # 2*log2(e)*2^7
B_SCH = 16250.0
# (g, rb) chunks whose exp+row-sum runs on DVE instead of ACT.
# Measured: the main loop is PE-bound (~60us of matmul in a ~65us
# window), so mid-loop offloads don't shorten it. But the TAIL is an ACT
# serial drain (the last fills finish ~2 chunks ahead of ACT): offloading
# the second-to-last chunk lets ACT and DVE drain in parallel.
# Measured: every exp-offload variant was neutral-to-worse, both when the
# loop was PE-bound AND after bf16 inputs made it ACT-bound ({(1,5),(3,5)}
# drew 98.1us vs 96.0/96.2 without — the DVE's 1x PSUM read holds the ring
# slot longer than ACT would, stalling the fills).
V_CHUNKS = frozenset()

F32 = mybir.dt.float32
BF16 = mybir.dt.bfloat16
I16 = mybir.dt.int16
U32 = mybir.dt.uint32
AF = mybir.ActivationFunctionType
OP = mybir.AluOpType
AX = mybir.AxisListType

_CACHE = {}


def _broadcast_last(ap: bass.AP, n: int) -> bass.AP:
    """Append a stride-0 dim of size n to an AP (free-axis broadcast)."""
    return bass.AP(tensor=ap.tensor, offset=ap.offset, ap=[*ap.ap, [0, n]])


def _build_nc():
    nc = bacc.Bacc(
        "TRN2", target_bir_lowering=False, debug=False, num_devices=N_CORES
    )
    # z ships as bf16: halves the input DMA time and makes the square
    # STTs 2-byte eligible; the bf16 quantization of z adds ~1e-5 relative
    # error to the loss (validated; gate is 2e-2).
    z = nc.dram_tensor("z", [P, NROWS], BF16, kind="ExternalInput").ap()
    ident = nc.dram_tensor("ident", [P, P], F32, kind="ExternalInput").ap()
    out = nc.dram_tensor("loss8", [P, RB], F32, kind="ExternalOutput").ap()

    with tile.TileContext(nc) as tc, ExitStack() as ctx:
        zpool = ctx.enter_context(tc.tile_pool(name="zpool", bufs=SC))
        sqpool = ctx.enter_context(tc.tile_pool(name="sqpool", bufs=3))
        znpool = ctx.enter_context(tc.tile_pool(name="znpool", bufs=4))
        small = ctx.enter_context(tc.tile_pool(name="small", bufs=4))
        i16pool = ctx.enter_context(tc.tile_pool(name="i16pool", bufs=2))
        vdpool = ctx.enter_context(tc.tile_pool(name="vdpool", bufs=2))
        singles = ctx.enter_context(tc.tile_pool(name="singles", bufs=1))
        psum = ctx.enter_context(tc.tile_pool(name="psum", bufs=2, space="PSUM"))

        # Persistent transposed normalized representation: RT[d, n]
        rt = singles.tile([P, NROWS], BF16)

        Ssum = singles.tile([P, RB * G], F32)  # per (row, chunk) partial sums
        d8 = singles.tile([P, RB], F32)  # exp(2*self)
        p8 = singles.tile([P, RB], F32)  # exp(2*pair)

        # ---- normalization: 8 pipelined sub-chunks of 1024 rows ----
        # All loads are emitted first so no queue-blocking wait (e.g. an
        # xbar transpose waiting on zn) can delay a later load's dispatch.
        zts = []
        for c in range(SC):
            zt = zpool.tile([P, TPS, D], BF16)
            if c == 0:
                # split the first (critical-path) load into two parallel DMAs
                half = SCR // 2
                # dispatch the critical first load from the Scalar hwdge
                # queue, which finishes engine startup before Sync does
                nc.scalar.dma_start(out=zt[:, : TPS // 2, :], in_=z[:, 0:half])
                nc.scalar.dma_start(out=zt[:, TPS // 2 :, :], in_=z[:, half:SCR])
            else:
                nc.sync.dma_start(out=zt[:], in_=z[:, c * SCR : (c + 1) * SCR])
            zts.append(zt)
        sb_ident = singles.tile([P, P], F32)
        nc.sync.dma_start(out=sb_ident[:], in_=ident)

        for c in range(SC):
            zt = zts[c]
            # fused square + row-sum: per tile one scalar_tensor_tensor with
            # accum_out (out = z*z is scratch, accum = sum over free axis).
            # (A fused 2-long-op variant -- one TT square + one reduce --
            # measured 2.7us SLOWER overall, and splitting sub-chunk 0 into
            # two 512-row units through the transpose measured 15us slower:
            # the fine-grained-but-uniform STT structure is what the
            # scheduler handles best.)
            sq = sqpool.tile([P, TPS, D], BF16)
            ss = small.tile([P, TPS], F32)
            zn = znpool.tile([P, TPS, D], BF16)
            for t in range(TPS):
                nc.vector.scalar_tensor_tensor(
                    out=sq[:, t, :],
                    in0=zt[:, t, :],
                    scalar=1.0,
                    in1=zt[:, t, :],
                    op0=OP.mult,
                    op1=OP.mult,
                    accum_out=ss[:, t : t + 1],
                )

            # u = 1/sqrt(ss).
            # NOTE: an ACT-Sqrt + DVE reciprocal_approx_fast variant for
            # c < 2 (shorter critical chain) drew {98.7, 101.0, 116.2}us
            # across three compiles -- the Tile scheduler is nondeterministic
            # between compiles and that variant is schedule-fragile. The
            # all-Quake form below drew a tight {100.5..100.8}us over four
            # compiles; we ship the low-variance config.
            u = small.tile([P, TPS], F32)
            if c < 2:
                # Sub-chunks 0-1 gate the first matmuls (the whole g=0
                # column range), so their chain must be SHORT: ACT Sqrt
                # (engine idle in the prologue; its in-order queue puts all
                # Sqrts before the first Exp, so at most one extra table
                # load, paid while ACT is idle) + single-op DVE fast
                # reciprocal (~51 ULP) = 2 dependent hops instead of the
                # 8-hop Quake chain whose ~0.5-0.9us/hop semaphore latency
                # delayed the first matmul to ~25us.
                nsq = small.tile([P, TPS], F32)
                nc.scalar.activation(out=nsq[:], in_=ss[:], func=AF.Sqrt)
                nc.vector.reciprocal_approx_fast(out=u[:], in_=nsq[:])
            else:
                # Quake seed + 2 fused Newton iterations on DVE. The scalar
                # ALU promotes to f32, so build the seed as
                # (0xBE6EB3BE - bits) via float mult/add, then integer >>1.
                tmp = small.tile([P, TPS], F32)
                nc.vector.tensor_scalar(
                    tmp[:].bitcast(U32),
                    ss[:].bitcast(U32),
                    -1.0,
                    float(0xBE6EB3BE),
                    OP.mult,
                    OP.add,
                )
                nc.vector.tensor_scalar(
                    u[:].bitcast(U32),
                    tmp[:].bitcast(U32),
                    1,
                    None,
                    OP.logical_shift_right,
                )
                for _ in range(2):
                    # t = (y*y * -0.5) * ss ; y = (t + 1.5) * y
                    nc.vector.tensor_mul(tmp[:], u[:], u[:])
                    nc.vector.scalar_tensor_tensor(
                        out=tmp[:], in0=tmp[:], scalar=-0.5, in1=ss[:],
                        op0=OP.mult, op1=OP.mult,
                    )
                    nc.vector.scalar_tensor_tensor(
                        out=u[:], in0=tmp[:], scalar=1.5, in1=u[:],
                        op0=OP.add, op1=OP.mult,
                    )

            nc.vector.tensor_mul(zn[:], zt[:], _broadcast_last(u[:], D))
            # batched xbar transpose for the whole 1024-col sub-chunk:
            # out[a, b, c] = in[c, b*128 + a]  ->  rt[d, t*128+p] = zn[p, t, d]
            nc.sync.dma_start(
                out=rt[:, c * SCR : (c + 1) * SCR].rearrange(
                    "d (t p) -> d t p", p=P
                ),
                in_=zn[:],
                transpose=True,
            )

        # ---- PE warm-up ----
        # The PE sits idle for the whole ~24us normalization prologue, and
        # the power governor then takes ~30us of wall time to ramp the
        # array from ~634ns to its sustained ~376ns per 512-col matmul
        # (~8us lost across the first ~60 real matmuls). Feed it dummy
        # fp32 ident x ident matmuls (ready as soon as the ident DMA lands
        # at ~8.5us) so the real stream starts warm. They finish before
        # rt sub-chunk 0 is transposed, so they delay nothing.
        for _ in range(28):
            pw = psum.tile([P, CH], F32, name="ps")
            nc.tensor.matmul(
                pw[:, 0:P], sb_ident[:], sb_ident[:], start=True, stop=True
            )

        # ---- main loop: sim chunk -> exp -> row-sum ----
        expool = ctx.enter_context(tc.tile_pool(name="expool", bufs=6))
        for g in range(G):
            for rb in range(RB):
                ps = psum.tile([P, CH], F32)
                for s in range(CH // MM_N):
                    nc.tensor.matmul(
                        ps[:, s * MM_N : (s + 1) * MM_N],
                        rt[:, rb * P : (rb + 1) * P],
                        rt[:, g * CH + s * MM_N : g * CH + (s + 1) * MM_N],
                        start=True,
                        stop=True,
                    )
                if g in (0, 2):
                    # chunks carrying the self/pair diagonal: exp to SBUF
                    # scratch (PSUM frees immediately; extraction decouples)
                    # (Dropping accum_out here and row-summing ex on the DVE
                    # instead -- to save the 16x300ns RD_ACC flushes on the
                    # ACT stream -- drew 99.7us vs 95.3: the DVE STTs ran 1x
                    # and swamped its mid-loop slack. Keep the ACT accum.)
                    ex = expool.tile([P, CH], F32)
                    nc.scalar.activation(
                        out=ex[:],
                        in_=ps[:],
                        func=AF.Exp,
                        scale=2.0,
                        accum_out=Ssum[:, rb * G + g : rb * G + g + 1],
                    )
                    # diag extract off the critical path: multiply the
                    # 128x128 diag block by identity, reduce along free
                    dst = (d8 if g == 0 else p8)[:, rb : rb + 1]
                    dummy = small.tile([P, P], F32)
                    nc.vector.tensor_mul(
                        dummy[:], ex[:, rb * P : rb * P + P], sb_ident[:]
                    )
                    nc.vector.tensor_reduce(dst, dummy[:], axis=AX.X, op=OP.add)
                elif (g, rb) in V_CHUNKS:
                    # DVE-offloaded exp: one tensor_scalar converts the
                    # whole chunk to bf16-bit exp2 int16s (frees the PSUM
                    # slot); the 2-byte row-sum runs after the main loop,
                    # hidden under the remaining ACT chunks.
                    it16 = i16pool.tile([P, CH], I16)
                    nc.vector.tensor_scalar(
                        it16[:], ps[:], A_SCH, B_SCH, OP.mult, OP.add
                    )
                    # row-sum via STT-with-accum ((x*0)+x): 2-byte fast
                    # path (TensorReduce always runs 1x). Inline: it reads
                    # SBUF, so the PSUM slot is already free, and the DVE
                    # has slack here.
                    vd = vdpool.tile([P, CH], BF16)
                    nc.vector.scalar_tensor_tensor(
                        out=vd[:], in0=it16[:].bitcast(BF16), scalar=0.0,
                        in1=it16[:].bitcast(BF16), op0=OP.mult, op1=OP.add,
                        accum_out=Ssum[:, rb * G + g : rb * G + g + 1],
                    )
                else:
                    nc.scalar.activation(
                        out=ps[:],
                        in_=ps[:],
                        func=AF.Exp,
                        scale=2.0,
                        accum_out=Ssum[:, rb * G + g : rb * G + g + 1],
                    )

        # ---- epilogue ----
        # (A split two-column-group variant with a fused ln(S'/p) -- meant
        # to overlap rb0-5's reduction chain with the ACT drain -- drew
        # {98.5, 99.7}us vs this form's {97.8, 97.8}us. Reverted.)
        S8 = singles.tile([P, RB], F32)
        nc.vector.tensor_reduce(
            S8[:], Ssum[:].rearrange("p (r g) -> p r g", g=G), axis=AX.X, op=OP.add
        )
        # S8 <- S8 - d8 + p8
        nc.vector.scalar_tensor_tensor(
            out=S8[:], in0=d8[:], scalar=-1.0, in1=S8[:], op0=OP.mult, op1=OP.add
        )
        nc.vector.tensor_add(S8[:], S8[:], p8[:])
        lse = singles.tile([P, RB], F32)
        nc.scalar.activation(out=lse[:], in_=S8[:], func=AF.Ln)
        p2 = singles.tile([P, RB], F32)
        nc.scalar.activation(out=p2[:], in_=p8[:], func=AF.Ln)
        loss8 = singles.tile([P, RB], F32)
        nc.vector.scalar_tensor_tensor(
            out=loss8[:], in0=p2[:], scalar=-1.0, in1=lse[:], op0=OP.mult, op1=OP.add
        )
        nc.sync.dma_start(out=out, in_=loss8[:])

    nc.compile()
    return nc


def get_nc():
    if "nc" not in _CACHE:
        _CACHE["nc"] = _build_nc()
    return _CACHE["nc"]


def make_in_maps(z_i: np.ndarray, z_j: np.ndarray):
    Z = np.concatenate(
        [
            np.asarray(z_i, np.float32).reshape(NROWS // 2, D),
            np.asarray(z_j, np.float32).reshape(NROWS // 2, D),
        ],
        axis=0,
    )
    ident = np.eye(P, dtype=np.float32)
    in_maps = []
    for k in range(N_CORES):
        zk = np.roll(Z, -k * ROWS_PER_CORE, axis=0)
        # z_sh[p, t*128+d] = zk[t*128+p, d]: contiguous per-partition DMA
        zsh = np.ascontiguousarray(
            zk.reshape(NROWS // P, P, D).transpose(1, 0, 2)
        ).reshape(P, NROWS).astype(BF16_NP)
        in_maps.append({"z": zsh, "ident": ident})
    return in_maps


def run_full(z_i: np.ndarray, z_j: np.ndarray, trace: bool = False):
    nc = get_nc()
    in_maps = make_in_maps(z_i, z_j)
    res = run_bass_kernel_spmd(nc, in_maps, list(range(N_CORES)), trace=trace)
    total = 0.0
    for k in range(N_CORES):
        total += float(np.asarray(res.results[k]["loss8"], np.float64).sum())
    loss = np.float32(total / NROWS)
    return loss, res


def kernel(z_i: np.ndarray, z_j: np.ndarray) -> np.ndarray:
    loss, _ = run_full(z_i, z_j, trace=False)
    return np.asarray(loss, dtype=np.float32)


# revision 81
# speedup vs baseline: 1.0558x; 1.0085x over previous
"""NT-Xent loss kernel for 8 Trainium2 NeuronCores.

Math (matches the reference):
  Z = concat(z_i, z_j).reshape(8192, 128); r = row-l2-normalize(Z)
  sim = r @ r.T                                  (8192 x 8192)
  row i: S_i   = sum_j exp(2*sim[i, j])          (full row, incl. self)
         d_i   = exp(2*sim[i, i])                (self term)
         p_i   = exp(2*sim[i, pair(i)]),  pair(i) = (i + 4096) % 8192
  loss_i = log(S_i - d_i + p_i) - log(p_i)
  loss   = mean_i(loss_i)

Sharding: rows are split across 8 cores (1024 rows each). Every core gets
the full Z, but ROTATED so its own rows come first; this makes the
self-diagonal land at local columns [0, 1024) and the pair diagonal at
[4096, 5120) on every core, so one SPMD program works for all cores.
Each core emits its 1024 per-row losses; the host sums them (the scalar
all-reduce) and divides by 2N.

Host-side staging: z is supplied pre-rotated AND pre-tiled as
z_sh[p, t*128 + d] = z_rot[t*128 + p, d] so every DMA is fully
contiguous per partition.

Per-core pipeline:
  1. DMA z (fp32) in 8 sub-chunks of 1024 rows.
  2. Row norms on DVE only: square, reduce, rsqrt via Quake seed + 2
     fused Newton steps (no ACT Sqrt -> no activation-table thrash).
  3. Scale rows to unit norm, cast bf16, one batched DMA-xbar transpose
     per sub-chunk into RT[d, 8192].
  4. Main loop: per (2048-col chunk x 128-row block): 4 bf16 matmuls
     (512 cols each) into PSUM, one ACT Exp (scale=2) with fused row-sum
     (accum_out); diag-carrying chunks write exp to SBUF scratch so the
     PSUM slot frees immediately and the diag extraction (multiply by
     identity + reduce on DVE) runs off the critical path.
     Four late non-diag chunks are offloaded from the saturated ACT
     engine to the (by then idle) DVE via a one-pass Schraudolph
     exp2-in-int16 (bf16 bit pattern) + fast 2-byte row-sum; the ~3%
     element error is mean-centered and contributes < 2e-4 to the loss.
  5. Epilogue: S - d + p, Ln, subtract, DMA out [128, 8] losses.
"""

import sys

import numpy as np

sys.path.insert(0, "/opt/trn_rl_repo")

from contextlib import ExitStack  # noqa: E402

import concourse.bass as bass  # noqa: E402
import concourse.tile as tile  # noqa: E402
from concourse import bacc, mybir  # noqa: E402
from concourse.bass_utils import run_bass_kernel_spmd  # noqa: E402

try:
    import ml_dtypes  # noqa: E402

    BF16_NP = ml_dtypes.bfloat16
except ImportError:  # pragma: no cover
    BF16_NP = np.float32

P = 128
N_CORES = 8
NROWS = 8192  # 2N
D = 128
ROWS_PER_CORE = NROWS // N_CORES  # 1024
RB = ROWS_PER_CORE // P  # 8 row blocks per core
G = 4  # column chunk groups (main loop)
CH = NROWS // G  # 2048 columns per chunk
SC = 8  # normalization sub-chunks
SCR = NROWS // SC  # 1024 rows per sub-chunk
TPS = SCR // P  # 8 row-tiles per sub-chunk
MM_N = 512  # matmul moving free dim (one PSUM bank)

# Schraudolph exp2-in-bf16-bits for the DVE-offloaded chunks:
# int16 = sim*A + B; the bits, read as bf16, give exp(2*sim) with ~3%
# element error, mean-centered (B tuned on the real input distribution;
# validated |rel err| of the final loss < 1.3e-4).
A_SCH = 2.0 * 128.0 * 1.4426950408889634  # 2*log2(e)*2^7
B_SCH = 16250.0
# (g, rb) chunks whose exp+row-sum runs on DVE instead of ACT.
# Measured: the main loop is PE-bound (~60us of matmul in a ~65us
# window), so mid-loop offloads don't shorten it. But the TAIL is an ACT
# serial drain (the last fills finish ~2 chunks ahead of ACT): offloading
# the second-to-last chunk lets ACT and DVE drain in parallel.
# Measured: every exp-offload variant was neutral-to-worse, both when the
# loop was PE-bound AND after bf16 inputs made it ACT-bound ({(1,5),(3,5)}
# drew 98.1us vs 96.0/96.2 without — the DVE's 1x PSUM read holds the ring
# slot longer than ACT would, stalling the fills).
V_CHUNKS = frozenset()

F32 = mybir.dt.float32
BF16 = mybir.dt.bfloat16
I16 = mybir.dt.int16
U32 = mybir.dt.uint32
AF = mybir.ActivationFunctionType
OP = mybir.AluOpType
AX = mybir.AxisListType

_CACHE = {}


def _broadcast_last(ap: bass.AP, n: int) -> bass.AP:
    """Append a stride-0 dim of size n to an AP (free-axis broadcast)."""
    return bass.AP(tensor=ap.tensor, offset=ap.offset, ap=[*ap.ap, [0, n]])


def _build_nc():
    nc = bacc.Bacc(
        "TRN2", target_bir_lowering=False, debug=False, num_devices=N_CORES
    )
    # z ships as bf16: halves the input DMA time and makes the square
    # STTs 2-byte eligible; the bf16 quantization of z adds ~1e-5 relative
    # error to the loss (validated; gate is 2e-2).
    z = nc.dram_tensor("z", [P, NROWS], BF16, kind="ExternalInput").ap()
    ident = nc.dram_tensor("ident", [P, P], F32, kind="ExternalInput").ap()
    out = nc.dram_tensor("loss8", [P, RB], F32, kind="ExternalOutput").ap()

    with tile.TileContext(nc) as tc, ExitStack() as ctx:
        zpool = ctx.enter_context(tc.tile_pool(name="zpool", bufs=SC))
        sqpool = ctx.enter_context(tc.tile_pool(name="sqpool", bufs=3))
        znpool = ctx.enter_context(tc.tile_pool(name="znpool", bufs=4))
        small = ctx.enter_context(tc.tile_pool(name="small", bufs=4))
        i16pool = ctx.enter_context(tc.tile_pool(name="i16pool", bufs=2))
        vdpool = ctx.enter_context(tc.tile_pool(name="vdpool", bufs=2))
        singles = ctx.enter_context(tc.tile_pool(name="singles", bufs=1))
        psum = ctx.enter_context(tc.tile_pool(name="psum", bufs=2, space="PSUM"))

        # Persistent transposed normalized representation: RT[d, n]
        rt = singles.tile([P, NROWS], BF16)

        Ssum = singles.tile([P, RB * G], F32)  # per (row, chunk) partial sums
        d8 = singles.tile([P, RB], F32)  # exp(2*self)
        p8 = singles.tile([P, RB], F32)  # exp(2*pair)

        # ---- normalization: 8 pipelined sub-chunks of 1024 rows ----
        # All loads are emitted first so no queue-blocking wait (e.g. an
        # xbar transpose waiting on zn) can delay a later load's dispatch.
        zts = []
        for c in range(SC):
            zt = zpool.tile([P, TPS, D], BF16)
            if c == 0:
                # split the first (critical-path) load into two parallel DMAs
                half = SCR // 2
                # dispatch the critical first load from the Scalar hwdge
                # queue, which finishes engine startup before Sync does
                nc.scalar.dma_start(out=zt[:, : TPS // 2, :], in_=z[:, 0:half])
                nc.scalar.dma_start(out=zt[:, TPS // 2 :, :], in_=z[:, half:SCR])
            else:
                nc.sync.dma_start(out=zt[:], in_=z[:, c * SCR : (c + 1) * SCR])
            zts.append(zt)
        sb_ident = singles.tile([P, P], F32)
        nc.sync.dma_start(out=sb_ident[:], in_=ident)

        for c in range(SC):
            zt = zts[c]
            # fused square + row-sum: per tile one scalar_tensor_tensor with
            # accum_out (out = z*z is scratch, accum = sum over free axis).
            # (A fused 2-long-op variant -- one TT square + one reduce --
            # measured 2.7us SLOWER overall, and splitting sub-chunk 0 into
            # two 512-row units through the transpose measured 15us slower:
            # the fine-grained-but-uniform STT structure is what the
            # scheduler handles best.)
            sq = sqpool.tile([P, TPS, D], BF16)
            ss = small.tile([P, TPS], F32)
            zn = znpool.tile([P, TPS, D], BF16)
            for t in range(TPS):
                nc.vector.scalar_tensor_tensor(
                    out=sq[:, t, :],
                    in0=zt[:, t, :],
                    scalar=1.0,
                    in1=zt[:, t, :],
                    op0=OP.mult,
                    op1=OP.mult,
                    accum_out=ss[:, t : t + 1],
                )

            # u = 1/sqrt(ss).
            # NOTE: an ACT-Sqrt + DVE reciprocal_approx_fast variant for
            # c < 2 (shorter critical chain) drew {98.7, 101.0, 116.2}us
            # across three compiles -- the Tile scheduler is nondeterministic
            # between compiles and that variant is schedule-fragile. The
            # all-Quake form below drew a tight {100.5..100.8}us over four
            # compiles; we ship the low-variance config.
            u = small.tile([P, TPS], F32)
            if c < 2:
                # Sub-chunks 0-1 gate the first matmuls (the whole g=0
                # column range), so their chain must be SHORT: ACT Sqrt
                # (engine idle in the prologue; its in-order queue puts all
                # Sqrts before the first Exp, so at most one extra table
                # load, paid while ACT is idle) + single-op DVE fast
                # reciprocal (~51 ULP) = 2 dependent hops instead of the
                # 8-hop Quake chain whose ~0.5-0.9us/hop semaphore latency
                # delayed the first matmul to ~25us.
                nsq = small.tile([P, TPS], F32)
                nc.scalar.activation(out=nsq[:], in_=ss[:], func=AF.Sqrt)
                nc.vector.reciprocal_approx_fast(out=u[:], in_=nsq[:])
            else:
                # Quake seed + 2 fused Newton iterations on DVE. The scalar
                # ALU promotes to f32, so build the seed as
                # (0xBE6EB3BE - bits) via float mult/add, then integer >>1.
                tmp = small.tile([P, TPS], F32)
                nc.vector.tensor_scalar(
                    tmp[:].bitcast(U32),
                    ss[:].bitcast(U32),
                    -1.0,
                    float(0xBE6EB3BE),
                    OP.mult,
                    OP.add,
                )
                nc.vector.tensor_scalar(
                    u[:].bitcast(U32),
                    tmp[:].bitcast(U32),
                    1,
                    None,
                    OP.logical_shift_right,
                )
                for _ in range(2):
                    # t = (y*y * -0.5) * ss ; y = (t + 1.5) * y
                    nc.vector.tensor_mul(tmp[:], u[:], u[:])
                    nc.vector.scalar_tensor_tensor(
                        out=tmp[:], in0=tmp[:], scalar=-0.5, in1=ss[:],
                        op0=OP.mult, op1=OP.mult,
                    )
                    nc.vector.scalar_tensor_tensor(
                        out=u[:], in0=tmp[:], scalar=1.5, in1=u[:],
                        op0=OP.add, op1=OP.mult,
                    )

            nc.vector.tensor_mul(zn[:], zt[:], _broadcast_last(u[:], D))
            # batched xbar transpose for the whole 1024-col sub-chunk:
            # out[a, b, c] = in[c, b*128 + a]  ->  rt[d, t*128+p] = zn[p, t, d]
            nc.sync.dma_start(
                out=rt[:, c * SCR : (c + 1) * SCR].rearrange(
                    "d (t p) -> d t p", p=P
                ),
                in_=zn[:],
                transpose=True,
            )

        # ---- PE warm-up ----
        # The PE sits idle for the whole ~24us normalization prologue, and
        # the power governor then takes ~30us of wall time to ramp the
        # array from ~634ns to its sustained ~376ns per 512-col matmul
        # (~8us lost across the first ~60 real matmuls). Feed it dummy
        # fp32 ident x ident matmuls (ready as soon as the ident DMA lands
        # at ~8.5us) so the real stream starts warm. They finish before
        # rt sub-chunk 0 is transposed, so they delay nothing.
        for _ in range(28):
            pw = psum.tile([P, CH], F32, name="ps")
            nc.tensor.matmul(
                pw[:, 0:P], sb_ident[:], sb_ident[:], start=True, stop=True
            )

        # ---- main loop: sim chunk -> exp -> row-sum ----
        expool = ctx.enter_context(tc.tile_pool(name="expool", bufs=6))
        for g in range(G):
            for rb in range(RB):
                ps = psum.tile([P, CH], F32)
                for s in range(CH // MM_N):
                    nc.tensor.matmul(
                        ps[:, s * MM_N : (s + 1) * MM_N],
                        rt[:, rb * P : (rb + 1) * P],
                        rt[:, g * CH + s * MM_N : g * CH + (s + 1) * MM_N],
                        start=True,
                        stop=True,
                    )
                if g in (0, 2):
                    # chunks carrying the self/pair diagonal: exp to SBUF
                    # scratch (PSUM frees immediately; extraction decouples)
                    # (Dropping accum_out here and row-summing ex on the DVE
                    # instead -- to save the 16x300ns RD_ACC flushes on the
                    # ACT stream -- drew 99.7us vs 95.3: the DVE STTs ran 1x
                    # and swamped its mid-loop slack. Keep the ACT accum.)
                    ex = expool.tile([P, CH], F32)
                    nc.scalar.activation(
                        out=ex[:],
                        in_=ps[:],
                        func=AF.Exp,
                        scale=2.0,
                        accum_out=Ssum[:, rb * G + g : rb * G + g + 1],
                    )
                    # diag extract off the critical path: multiply the
                    # 128x128 diag block by identity, reduce along free
                    dst = (d8 if g == 0 else p8)[:, rb : rb + 1]
                    dummy = small.tile([P, P], F32)
                    nc.vector.tensor_mul(
                        dummy[:], ex[:, rb * P : rb * P + P], sb_ident[:]
                    )
                    nc.vector.tensor_reduce(dst, dummy[:], axis=AX.X, op=OP.add)
                elif (g, rb) in V_CHUNKS:
                    # DVE-offloaded exp: one tensor_scalar converts the
                    # whole chunk to bf16-bit exp2 int16s (frees the PSUM
                    # slot); the 2-byte row-sum runs after the main loop,
                    # hidden under the remaining ACT chunks.
                    it16 = i16pool.tile([P, CH], I16)
                    nc.vector.tensor_scalar(
                        it16[:], ps[:], A_SCH, B_SCH, OP.mult, OP.add
                    )
                    # row-sum via STT-with-accum ((x*0)+x): 2-byte fast
                    # path (TensorReduce always runs 1x). Inline: it reads
                    # SBUF, so the PSUM slot is already free, and the DVE
                    # has slack here.
                    vd = vdpool.tile([P, CH], BF16)
                    nc.vector.scalar_tensor_tensor(
                        out=vd[:], in0=it16[:].bitcast(BF16), scalar=0.0,
                        in1=it16[:].bitcast(BF16), op0=OP.mult, op1=OP.add,
                        accum_out=Ssum[:, rb * G + g : rb * G + g + 1],
                    )
                else:
                    nc.scalar.activation(
                        out=ps[:],
                        in_=ps[:],
                        func=AF.Exp,
                        scale=2.0,
                        accum_out=Ssum[:, rb * G + g : rb * G + g + 1],
                    )

        # ---- epilogue ----
        # (A split two-column-group variant with a fused ln(S'/p) -- meant
        # to overlap rb0-5's reduction chain with the ACT drain -- drew
        # {98.5, 99.7}us vs this form's {97.8, 97.8}us. Reverted.)
        S8 = singles.tile([P, RB], F32)
        nc.vector.tensor_reduce(
            S8[:], Ssum[:].rearrange("p (r g) -> p r g", g=G), axis=AX.X, op=OP.add
        )
        # S8 <- S8 - d8 + p8
        nc.vector.scalar_tensor_tensor(
            out=S8[:], in0=d8[:], scalar=-1.0, in1=S8[:], op0=OP.mult, op1=OP.add
        )
        nc.vector.tensor_add(S8[:], S8[:], p8[:])
        # loss = ln(S'/p): the bit-exact reciprocal is ready as soon as the
        # g=2 extracts finish (well before the last ACT chunk), so the
        # post-drain tail is reduce -> STT -> add -> mul -> ONE Ln -> DMA
        # instead of paying a second Ln + subtract after the table swap.
        rp8 = singles.tile([P, RB], F32)
        nc.vector.reciprocal(rp8[:], p8[:])
        nc.vector.tensor_mul(S8[:], S8[:], rp8[:])
        loss8 = singles.tile([P, RB], F32)
        nc.scalar.activation(out=loss8[:], in_=S8[:], func=AF.Ln)
        nc.sync.dma_start(out=out, in_=loss8[:])

    nc.compile()
    return nc


def get_nc():
    if "nc" not in _CACHE:
        _CACHE["nc"] = _build_nc()
    return _CACHE["nc"]


def make_in_maps(z_i: np.ndarray, z_j: np.ndarray):
    Z = np.concatenate(
        [
            np.asarray(z_i, np.float32).reshape(NROWS // 2, D),
            np.asarray(z_j, np.float32).reshape(NROWS // 2, D),
        ],
        axis=0,
    )
    ident = np.eye(P, dtype=np.float32)
    in_maps = []
    for k in range(N_CORES):
        zk = np.roll(Z, -k * ROWS_PER_CORE, axis=0)
        # z_sh[p, t*128+d] = zk[t*128+p, d]: contiguous per-partition DMA
        zsh = np.ascontiguousarray(
            zk.reshape(NROWS // P, P, D).transpose(1, 0, 2)
        ).reshape(P, NROWS).astype(BF16_NP)
        in_maps.append({"z": zsh, "ident": ident})
    return in_maps


def run_full(z_i: np.ndarray, z_j: np.ndarray, trace: bool = False):
    nc = get_nc()
    in_maps = make_in_maps(z_i, z_j)
    res = run_bass_kernel_spmd(nc, in_maps, list(range(N_CORES)), trace=trace)
    total = 0.0
    for k in range(N_CORES):
        total += float(np.asarray(res.results[k]["loss8"], np.float64).sum())
    loss = np.float32(total / NROWS)
    return loss, res


def kernel(z_i: np.ndarray, z_j: np.ndarray) -> np.ndarray:
    loss, _ = run_full(z_i, z_j, trace=False)
    return np.asarray(loss, dtype=np.float32)
